# revision 1
# baseline (speedup 1.0000x reference)
"""Trainium2 Bass kernel for nn_MinimalNetwork (equivariant GNN message passing).

Fully fused per-edge pipeline, sharded over 8 NeuronCores by edge (data
parallel). Each core:
  radial-basis -> 3-layer silu MLP -> R [e,1216] (TensorE, PSUM-resident)
  CY = rsh @ CC2 (CG coefficients folded into one constant matmul)
  D-stage: 72 per-edge-scalar MACs (VectorE scalar_tensor_tensor)
  R-stage: 9 broadcast products + halving-tree (v,k) reduction (VectorE)
  scatter: DMA-CCE scatter-add of per-edge messages into the node table.
Host sums the 8 per-core node tables.

Self-contained: all shapes/layouts hardcoded for the 200000-edge / 12500-node
problem instance (works for any multiple-of-512 edge shard; see kernel()).
"""

import math
from contextlib import ExitStack
from itertools import accumulate

import numpy as np

# ----------------- problem constants (hardcoded) -----------------
N_NODES = 12500
N_EDGES = 200000
N_CORES = 8
RS = [(8, 0), (8, 1), (8, 2)]
LO = [0, 1, 2]
SH_DIM = 25
FEAT_OFF = [0] + list(accumulate(m * (2 * l + 1) for m, l in RS))
FEAT_DIM = FEAT_OFF[-1]  # 72
R_OFF = [0] + list(
    accumulate(mo * mi * (2 * min(lo, li) + 1) for mo, lo in RS for mi, li in RS)
)
R_DIM = R_OFF[-1]  # 1216
N_BASIS, H = 10, 100
MIN_R, MAX_R = 0.7, 3.2
SWISH_SCALE = 1.679177
SUB = 128          # edges per sub-tile (partition dim)
SUPER = 512        # edges per super-tile (MLP batch)
N_SUB = SUPER // SUB


def _pair_nl(i, j):
    return 2 * min(LO[i], LO[j]) + 1


def _wj(j):
    return sum(_pair_nl(i, j) * (2 * LO[i] + 1) for i in range(3))


W_J = [_wj(j) for j in range(3)]  # [9, 25, 35]


def _off_ij(i, j):
    return sum(_pair_nl(i2, j) * (2 * LO[i2] + 1) for i2 in range(i))


# uniform padded D layout: every j-block is [v(8) x WU(35)], stride 280
WU = max(W_J)
D_JOFF = [8 * WU * j for j in range(4)]
D_DIM = D_JOFF[-1]  # 840
CY_JOFF = [0] + list(accumulate((2 * LO[j] + 1) * W_J[j] for j in range(3)))
CY_DIM = CY_JOFF[-1]  # 259


def _cc_layout():
    layout, off = {}, 0
    for _, lo in RS:
        for _, li in RS:
            for lf in range(abs(lo - li), lo + li + 1):
                if (lo, li, lf) not in layout:
                    shp = (2 * lo + 1, 2 * li + 1, 2 * lf + 1)
                    layout[(lo, li, lf)] = (off, shp)
                    off += shp[0] * shp[1] * shp[2]
    return layout, off


CC_LAYOUT, CC_TOTAL = _cc_layout()  # 1225


def _norm_coef():
    nc = np.zeros((3, 3), dtype=np.float64)
    for i, (_, lo) in enumerate(RS):
        ns = sum(mi * (2 * min(lo, li) + 1) for mi, li in RS)
        nc[i, :] = math.sqrt(4 * math.pi) * math.sqrt(2 * lo + 1) / math.sqrt(ns)
    return nc


NORM = _norm_coef()


def build_cc2(cc: np.ndarray) -> np.ndarray:
    """CC2 [25, CY_DIM]: CY[e,:] = rsh[e,:] @ CC2 (NORM folded in)."""
    cc2 = np.zeros((SH_DIM, CY_DIM), dtype=np.float32)
    for j in range(3):
        lj = LO[j]
        for ii in range(2 * lj + 1):
            blk = CY_JOFF[j] + ii * W_J[j]
            for i in range(3):
                lo = LO[i]
                no = 2 * lo + 1
                base = blk + _off_ij(i, j)
                for k, lf in enumerate(range(abs(lo - lj), lo + lj + 1)):
                    off, shp = CC_LAYOUT[(lo, lj, lf)]
                    C = cc[off : off + shp[0] * shp[1] * shp[2]].reshape(shp)
                    for o in range(no):
                        col = base + k * no + o
                        cc2[lf * lf : lf * lf + 2 * lf + 1, col] = (
                            np.float32(NORM[i, j]) * C[o, ii, :]
                        )
    return cc2


def fold_weights(W0, W1, W2, W3):
    s = SWISH_SCALE
    return (
        (W0 / math.sqrt(N_BASIS)).astype(np.float32),
        (s * W1 / math.sqrt(H)).astype(np.float32),
        (s * W2 / math.sqrt(H)).astype(np.float32),
        (s * W3 / math.sqrt(H)).astype(np.float32),
    )


# ----------------- bass program -----------------

def build_program(e_pad: int, n_nodes: int):
    import concourse.bass as bass
    import concourse.tile as tile
    from concourse import bacc, mybir

    f32 = mybir.dt.float32
    i32 = mybir.dt.int32
    AF = mybir.ActivationFunctionType
    OP = mybir.AluOpType

    n_super = e_pad // SUPER
    assert e_pad % SUPER == 0

    nc = bacc.Bacc()

    # DRAM tensors (per-core inputs)
    rshT_d = nc.dram_tensor("rshT", [SH_DIM, e_pad], f32, kind="ExternalInput")
    radii_d = nc.dram_tensor("radii", [1, e_pad], f32, kind="ExternalInput")
    src_d = nc.dram_tensor("srcidx", [e_pad, 1], i32, kind="ExternalInput")
    dst_d = nc.dram_tensor("dstidx", [e_pad, 1], i32, kind="ExternalInput")
    dstf_d = nc.dram_tensor("dstf", [e_pad, 1], f32, kind="ExternalInput")
    feat_d = nc.dram_tensor("features", [n_nodes, FEAT_DIM], f32, kind="ExternalInput")
    w0_d = nc.dram_tensor("W0p", [N_BASIS, H], f32, kind="ExternalInput")
    w1_d = nc.dram_tensor("W1p", [H, H], f32, kind="ExternalInput")
    w2_d = nc.dram_tensor("W2p", [H, H], f32, kind="ExternalInput")
    w3_d = nc.dram_tensor("W3p", [H, R_DIM], f32, kind="ExternalInput")
    cc2_d = nc.dram_tensor("CC2", [SH_DIM, CY_DIM], f32, kind="ExternalInput")
    csc_d = nc.dram_tensor("cscale", [N_BASIS, 1], f32, kind="ExternalInput")
    cbi_d = nc.dram_tensor("cbias", [N_BASIS, 1], f32, kind="ExternalInput")
    # +1 dummy row: pad edges scatter zeros there
    out_d = nc.dram_tensor("out", [n_nodes + 1, FEAT_DIM], f32, kind="ExternalOutput")

    with tile.TileContext(nc) as tc, ExitStack() as ctx:
        cpool = ctx.enter_context(tc.tile_pool(name="consts", bufs=1))
        inpool = ctx.enter_context(tc.tile_pool(name="in", bufs=3))
        hpool = ctx.enter_context(tc.tile_pool(name="h", bufs=2))
        dpool = ctx.enter_context(tc.tile_pool(name="work", bufs=2))
        rtpool = ctx.enter_context(tc.tile_pool(name="rtmp", bufs=2))
        mpool = ctx.enter_context(tc.tile_pool(name="msg", bufs=3))
        ps_mlp = ctx.enter_context(tc.tile_pool(name="psmlp", bufs=1, space="PSUM"))
        ps_r = ctx.enter_context(tc.tile_pool(name="psr", bufs=1, space="PSUM"))
        ps_cy = ctx.enter_context(tc.tile_pool(name="pscy", bufs=1, space="PSUM"))
        ps_bc = ctx.enter_context(tc.tile_pool(name="psbc", bufs=1, space="PSUM"))
        ps_cmb = ctx.enter_context(tc.tile_pool(name="pscmb", bufs=1, space="PSUM"))

        # constants into SBUF
        w0_s = cpool.tile([N_BASIS, H], f32)
        w1_s = cpool.tile([H, H], f32)
        w2_s = cpool.tile([H, H], f32)
        w3_s = cpool.tile([H, R_DIM], f32)
        cc2_s = cpool.tile([SH_DIM, CY_DIM], f32)
        csc_s = cpool.tile([N_BASIS, 1], f32)
        cbi_s = cpool.tile([N_BASIS, 1], f32)
        ones_s = cpool.tile([1, N_BASIS], f32)
        zero_s = cpool.tile([SUB, FEAT_DIM], f32)
        ident_s = cpool.tile([SUB, SUB], f32)
        for t, d in (
            (w0_s, w0_d), (w1_s, w1_d), (w2_s, w2_d), (w3_s, w3_d),
            (cc2_s, cc2_d), (csc_s, csc_d), (cbi_s, cbi_d),
        ):
            nc.sync.dma_start(t[:], d[:])
        nc.vector.memset(ones_s[:], 1.0)
        nc.vector.memset(zero_s[:], 0.0)
        from concourse.masks import make_identity
        make_identity(nc, ident_s[:])

        # zero-init the output table (n_nodes + 1 rows)
        n_out = n_nodes + 1
        nfull = n_out // SUB
        if nfull:
            nc.sync.dma_start(
                out_d[: nfull * SUB, :].rearrange("(a p) c -> p a c", p=SUB),
                zero_s[:].unsqueeze(1).broadcast_to((SUB, nfull, FEAT_DIM)),
            )
        rem = n_out - nfull * SUB
        if rem:
            nc.sync.dma_start(out_d[nfull * SUB :, :], zero_s[:rem, :])

        for s in range(n_super):
            e0 = s * SUPER
            # ---- loads ----
            rsh_t = inpool.tile([SH_DIM, SUPER], f32, tag="rsh")
            nc.sync.dma_start(rsh_t[:], rshT_d[:, e0 : e0 + SUPER])
            rad_t = inpool.tile([1, SUPER], f32, tag="rad")
            nc.sync.dma_start(rad_t[:], radii_d[:, e0 : e0 + SUPER])
            src_t = inpool.tile([SUB, N_SUB], i32, tag="src")
            nc.sync.dma_start(
                src_t[:],
                src_d[e0 : e0 + SUPER, 0].rearrange("(c p) -> p c", p=SUB),
            )
            dst_t = inpool.tile([SUB, N_SUB], i32, tag="dst")
            nc.sync.dma_start(
                dst_t[:],
                dst_d[e0 : e0 + SUPER, 0].rearrange("(c p) -> p c", p=SUB),
            )
            dstf_t = inpool.tile([SUB, N_SUB], f32, tag="dstf")
            nc.sync.dma_start(
                dstf_t[:],
                dstf_d[e0 : e0 + SUPER, 0].rearrange("(c p) -> p c", p=SUB),
            )
            fg_t = inpool.tile([SUB, N_SUB * FEAT_DIM], f32, tag="fg")
            for c in range(N_SUB):
                nc.gpsimd.indirect_dma_start(
                    out=fg_t[:, c * FEAT_DIM : (c + 1) * FEAT_DIM],
                    out_offset=None,
                    in_=feat_d[:],
                    in_offset=bass.IndirectOffsetOnAxis(ap=src_t[:, c : c + 1], axis=0),
                )

            # ---- radial basis ----
            rb_ps = ps_bc.tile([N_BASIS, SUPER], f32, tag="bc", space="PSUM")
            nc.tensor.matmul(rb_ps[:], ones_s[:], rad_t[:], start=True, stop=True)
            z2_t = hpool.tile([N_BASIS, SUPER], f32, tag="z2")
            nc.scalar.activation(
                z2_t[:], rb_ps[:], AF.Square, bias=cbi_s[:], scale=csc_s[:]
            )
            bas_t = hpool.tile([N_BASIS, SUPER], f32, tag="bas")
            nc.scalar.activation(bas_t[:], z2_t[:], AF.Exp, scale=-1.0)

            # ---- MLP ----
            h = bas_t
            for li, w_s in enumerate((w0_s, w1_s, w2_s)):
                hp = ps_mlp.tile([H, SUPER], f32, tag="hp", space="PSUM")
                nc.tensor.matmul(hp[:], w_s[:], h[:], start=True, stop=True)
                hn = hpool.tile([H, SUPER], f32, tag=f"h{li}")
                nc.scalar.activation(hn[:], hp[:], AF.Silu)
                h = hn

            for c in range(N_SUB):
                esl = slice(c * SUB, (c + 1) * SUB)
                # ---- R = h3_c^T @ W3p  -> PSUM [128, 1216] ----
                r_ps = ps_r.tile([SUB, R_DIM], f32, tag="r", space="PSUM")
                for n0 in range(0, R_DIM, 512):
                    n1 = min(n0 + 512, R_DIM)
                    nc.tensor.matmul(
                        r_ps[:, n0:n1], h[:, esl], w3_s[:, n0:n1],
                        start=True, stop=True,
                    )
                r_sb = dpool.tile([SUB, R_DIM], f32, tag="rsb")
                nc.scalar.copy(r_sb[:], r_ps[:])
                # ---- CY ----
                cy_ps = ps_cy.tile([SUB, CY_DIM], f32, tag="cy", space="PSUM")
                nc.tensor.matmul(
                    cy_ps[:], rsh_t[:, esl], cc2_s[:], start=True, stop=True
                )
                cy_t = dpool.tile([SUB, CY_DIM], f32, tag="cys")
                nc.scalar.copy(cy_t[:], cy_ps[:])

                # ---- D-stage: per j: one broadcast product + one reduce ----
                # Dtmp_j[v, w, ii] = F[(v,ii)] * CY[(ii, w)]; D_j[v, w] = sum_ii
                d_t = dpool.tile([SUB, D_DIM], f32, tag="d")
                for j in range(3):
                    nj = 2 * LO[j] + 1
                    w = W_J[j]
                    f_ap = (
                        fg_t[:, c * FEAT_DIM + FEAT_OFF[j] :
                             c * FEAT_DIM + FEAT_OFF[j + 1]]
                        .rearrange("p (v i) -> p v i", v=8)
                        .unsqueeze(3)
                        .broadcast_to((SUB, 8, nj, w))
                    )
                    cy_ap = (
                        cy_t[:, CY_JOFF[j] : CY_JOFF[j + 1]]
                        .rearrange("p (i w) -> p i w", i=nj)
                        .unsqueeze(1)
                        .broadcast_to((SUB, 8, nj, w))
                    )
                    dj = d_t[:, D_JOFF[j] : D_JOFF[j + 1]].rearrange(
                        "p (v w) -> p v w", w=WU
                    )[:, :, :w]
                    if nj == 1:
                        nc.vector.tensor_tensor(dj.unsqueeze(2), f_ap, cy_ap, OP.mult)
                    else:
                        dtmp = dpool.tile([SUB, 8 * w * nj], f32, tag=f"dt{j}")
                        # layout [v, w, ii] (ii innermost for the reduce);
                        # product iterates (v, ii, w)
                        out_ap = (
                            dtmp[:]
                            .rearrange("p (v w i) -> p v w i", v=8, w=w)
                            .transpose((0, 1, 3, 2))
                        )
                        nc.vector.tensor_tensor(out_ap, f_ap, cy_ap, OP.mult)
                        nc.vector.tensor_reduce(
                            dj,
                            dtmp[:].rearrange("p (v w i) -> p v w i", v=8, i=nj),
                            mybir.AxisListType.X,
                            OP.add,
                        )

                # ---- R-stage: per i: 3 products into joint [u, o, m] buffer,
                # one innermost-m reduce -> msg block ----
                msg_t = mpool.tile([SUB, FEAT_DIM], f32, tag="msg")
                for i in range(3):
                    lo = LO[i]
                    no = 2 * lo + 1
                    m_i = 8 * sum(_pair_nl(i, j) for j in range(3))
                    rt = rtpool.tile([SUB, 8 * no * m_i], f32, tag=f"rt{i}")
                    rt4 = rt[:].rearrange(
                        "p (u o m) -> p u o m", u=8, o=no
                    )
                    if i == 0:
                        # all three pairs have nl=1, no=1 and identical shapes:
                        # one joint product over (j, u, v)
                        r_ap = r_sb[:, R_OFF[0] : R_OFF[3]].rearrange(
                            "p (j u v) -> p j u v", j=3, u=8
                        )
                        d_ap = (
                            d_t[:]
                            .rearrange("p (j v w) -> p j v w", j=3, v=8)[:, :, :, 0]
                            .unsqueeze(2)
                            .broadcast_to((SUB, 3, 8, 8))
                        )
                        out_ap = rt[:].rearrange(
                            "p (u j v) -> p j u v", u=8, j=3
                        )
                        nc.vector.tensor_tensor(out_ap, r_ap, d_ap, OP.mult)
                        groups = []
                    elif i == 1:
                        groups = [(3, [0]), (4, [1]), (5, [2])]
                    else:
                        groups = [(6, [0]), (7, [1]), (8, [2])]
                    for p0, js in groups:
                        j = js[0]
                        nl = _pair_nl(i, j)
                        moff = 8 * sum(_pair_nl(i, j2) for j2 in range(j))
                        r_ap = (
                            r_sb[:, R_OFF[p0] : R_OFF[p0 + 1]]
                            .rearrange("p (u v k) -> p u v k", u=8, v=8)
                            .unsqueeze(4)
                            .broadcast_to((SUB, 8, 8, nl, no))
                        )
                        oij = _off_ij(i, j)
                        d_ap = (
                            d_t[:, D_JOFF[j] : D_JOFF[j + 1]]
                            .rearrange("p (v w) -> p v w", v=8)[
                                :, :, oij : oij + nl * no
                            ]
                            .rearrange("p v (k o) -> p v k o", k=nl)
                            .unsqueeze(1)
                            .broadcast_to((SUB, 8, 8, nl, no))
                        )
                        out_ap = (
                            rt4[:, :, :, moff : moff + 8 * nl]
                            .rearrange("p u o (v k) -> p u o v k", v=8)
                            .transpose((0, 1, 3, 4, 2))
                        )
                        nc.vector.tensor_tensor(out_ap, r_ap, d_ap, OP.mult)
                    mb = msg_t[:, FEAT_OFF[i] : FEAT_OFF[i + 1]]
                    nc.vector.tensor_reduce(
                        mb,
                        rt[:].rearrange("p (g m) -> p g m", m=m_i),
                        mybir.AxisListType.X,
                        OP.add,
                    )

                # ---- combine duplicate-dst rows, then scatter (plain write) ----
                # sel[p,q] = (dst[p] == dst[q]); msg2 = sel @ msg sums each
                # dst-group into every one of its rows, so colliding DMA
                # writes all carry identical values. Host guarantees a dst
                # never straddles a 128-edge tile.
                tp_ps = ps_cmb.tile([SUB, SUB], f32, tag="tp", space="PSUM")
                nc.tensor.transpose(
                    tp_ps[:],
                    dstf_t[:, c : c + 1].to_broadcast((SUB, SUB)),
                    ident_s[:],
                )
                dstT_t = mpool.tile([SUB, SUB], f32, tag="dstT")
                nc.scalar.copy(dstT_t[:], tp_ps[:])
                sel_t = mpool.tile([SUB, SUB], f32, tag="sel")
                nc.vector.tensor_tensor(
                    sel_t[:],
                    dstf_t[:, c : c + 1].to_broadcast((SUB, SUB)),
                    dstT_t[:],
                    OP.is_equal,
                )
                cmb_ps = ps_cmb.tile([SUB, FEAT_DIM], f32, tag="cmb", space="PSUM")
                nc.tensor.matmul(
                    cmb_ps[:], sel_t[:], msg_t[:], start=True, stop=True
                )
                msg2_t = mpool.tile([SUB, FEAT_DIM], f32, tag="msg2")
                nc.scalar.copy(msg2_t[:], cmb_ps[:])
                nc.gpsimd.indirect_dma_start(
                    out=out_d[:],
                    out_offset=bass.IndirectOffsetOnAxis(
                        ap=dst_t[:, c : c + 1], axis=0
                    ),
                    in_=msg2_t[:],
                    in_offset=None,
                )

    nc.finalize()
    return nc


# ----------------- host side -----------------

def _prep_consts(cc, W0, W1, W2, W3):
    W0p, W1p, W2p, W3p = fold_weights(W0, W1, W2, W3)
    cc2 = build_cc2(np.asarray(cc, dtype=np.float32))
    centers = np.linspace(MIN_R, MAX_R, N_BASIS).astype(np.float32)
    spacing = (MAX_R - MIN_R) / (N_BASIS - 1)
    cscale = np.full((N_BASIS, 1), 1.0 / spacing, dtype=np.float32)
    cbias = (-centers / spacing).astype(np.float32).reshape(N_BASIS, 1)
    return W0p, W1p, W2p, W3p, cc2, cscale, cbias


def pack_edges(dst: np.ndarray, n_nodes: int):
    """Group edges by dst and bin-pack the per-dst groups into 128-edge
    tiles (best-fit decreasing) so no dst's edge-group straddles a tile.
    Returns int64 array [n_tiles, SUB] of original edge ids, -1 for pads."""
    import bisect

    order = np.argsort(dst, kind="stable")
    ds = dst[order]
    starts = np.flatnonzero(np.r_[True, ds[1:] != ds[:-1]])
    ends = np.r_[starts[1:], len(ds)]
    runs = sorted(
        ((int(e - s), int(s), int(e)) for s, e in zip(starts, ends)),
        key=lambda r: -r[0],
    )
    assert runs[0][0] <= SUB, f"node with {runs[0][0]} > {SUB} in-edges"
    bins = []   # each: list of (s, e) sorted-run slices
    rems = []   # ascending remaining capacities, parallel with binidx
    binidx = []
    for L, s, e in runs:
        k = bisect.bisect_left(rems, L)
        if k == len(rems):
            bins.append([(s, e)])
            r, bi = SUB - L, len(bins) - 1
        else:
            bi = binidx[k]
            r = rems[k] - L
            del rems[k], binidx[k]
            bins[bi].append((s, e))
        j = bisect.bisect_left(rems, r)
        rems.insert(j, r)
        binidx.insert(j, bi)
    tiles = []
    for b in bins:
        cur = []
        for s, e in b:
            cur.extend(order[s:e].tolist())
        cur.extend([-1] * (SUB - len(cur)))
        tiles.append(cur)
    return np.array(tiles, dtype=np.int64)


def _build_and_maps(edge_index, features, radii, rsh, cc, W0, W1, W2, W3):
    edge_index = np.asarray(edge_index)
    features = np.ascontiguousarray(np.asarray(features, dtype=np.float32))
    radii = np.asarray(radii, dtype=np.float32)
    rsh = np.ascontiguousarray(np.asarray(rsh, dtype=np.float32))
    n_nodes = features.shape[0]
    E = radii.shape[0]

    W0p, W1p, W2p, W3p, cc2, cscale, cbias = _prep_consts(cc, W0, W1, W2, W3)

    src = edge_index[0].astype(np.int64)
    dst = edge_index[1].astype(np.int64)
    tiles = pack_edges(dst, n_nodes)
    n_tiles = tiles.shape[0]

    n_cores = N_CORES
    tiles_per_core = -(-n_tiles // n_cores)
    # round up to a whole number of super-tiles
    tpc = -(-tiles_per_core // N_SUB) * N_SUB
    e_pad = tpc * SUB

    nc = build_program(e_pad, n_nodes)

    in_maps = []
    for k in range(n_cores):
        sel = tiles[k * tiles_per_core : (k + 1) * tiles_per_core]
        flat = sel.reshape(-1)
        flat = np.concatenate([flat, np.full(e_pad - flat.size, -1, np.int64)])
        valid = flat >= 0
        idx = np.where(valid, flat, 0)

        rshT_s = np.ascontiguousarray(
            np.where(valid[None, :], rsh.T[:, idx], np.float32(0.0))
        ).astype(np.float32)
        radii_s = np.where(valid, radii[idx], np.float32(1.0)).reshape(1, -1)
        radii_s = np.ascontiguousarray(radii_s).astype(np.float32)
        src_s = np.where(valid, src[idx], 0).astype(np.int32).reshape(-1, 1)
        dst_v = np.where(valid, dst[idx], n_nodes)
        dst_s = dst_v.astype(np.int32).reshape(-1, 1)
        dstf_s = dst_v.astype(np.float32).reshape(-1, 1)
        in_maps.append(
            dict(
                rshT=rshT_s,
                radii=radii_s,
                srcidx=np.ascontiguousarray(src_s),
                dstidx=np.ascontiguousarray(dst_s),
                dstf=np.ascontiguousarray(dstf_s),
                features=features,
                W0p=W0p, W1p=W1p, W2p=W2p, W3p=W3p,
                CC2=cc2, cscale=cscale, cbias=cbias,
            )
        )

    return nc, in_maps, n_nodes


def kernel(edge_index, features, radii, rsh, cc, W0, W1, W2, W3):
    from concourse.bass_utils import run_bass_kernel_spmd

    nc, in_maps, n_nodes = _build_and_maps(
        edge_index, features, radii, rsh, cc, W0, W1, W2, W3
    )
    res = run_bass_kernel_spmd(nc, in_maps, core_ids=list(range(N_CORES)))
    out = np.zeros((n_nodes, FEAT_DIM), dtype=np.float32)
    for r in res.results:
        out += r["out"][:n_nodes]
    return out


def _install_ntff_shim():
    """Provide antenv.axon_hooks + the ctypes NTFF hook if absent."""
    import contextlib
    import ctypes
    import sys
    import types

    try:
        from antenv.axon_hooks import get_axon_ntff_profile_hook  # noqa: F401
        return
    except ImportError:
        pass

    holder = {}
    mod = types.ModuleType("antenv.axon_hooks")
    mod.set_axon_ntff_profile_hook = lambda h: holder.__setitem__("h", h)
    mod.get_axon_ntff_profile_hook = lambda: holder.get("h")
    import antenv

    sys.modules["antenv.axon_hooks"] = mod
    antenv.axon_hooks = mod

    so_path = "/opt/axon/libaxon_pjrt.so"
    try:
        lib = ctypes.CDLL(so_path)
    except OSError:
        return
    if not hasattr(lib, "axon_start_nrt_profile"):
        return
    lib.axon_start_nrt_profile.argtypes = [
        ctypes.POINTER(ctypes.c_int64),
        ctypes.c_size_t,
    ]
    lib.axon_start_nrt_profile.restype = ctypes.c_int64
    lib.axon_stop_nrt_profile.argtypes = [ctypes.c_char_p]
    lib.axon_stop_nrt_profile.restype = ctypes.c_int64

    @contextlib.contextmanager
    def _hook(output_dir, device_ids):
        import jax

        jax.devices()
        if device_ids:
            ids = (ctypes.c_int64 * len(device_ids))(*device_ids)
            rc = lib.axon_start_nrt_profile(ids, len(device_ids))
        else:
            rc = lib.axon_start_nrt_profile(None, 0)
        if rc != 0:
            raise RuntimeError(f"axon_start_nrt_profile rc={rc}")
        try:
            yield
        finally:
            n = lib.axon_stop_nrt_profile(str(output_dir).encode())
            print(f"ntff profile: {n} file(s) written to {output_dir}")

    mod.set_axon_ntff_profile_hook(_hook)


def kernel_traced(edge_index, features, radii, rsh, cc, W0, W1, W2, W3,
                  trace_cores=None, tmpdir=None):
    """Run with NTFF tracing; returns BassKernelResults."""
    _install_ntff_shim()
    from concourse import bass_utils

    # no artifact bucket in this container
    bass_utils.upload_artifacts = lambda d: f"local:{d}"

    nc, in_maps, n_nodes = _build_and_maps(
        edge_index, features, radii, rsh, cc, W0, W1, W2, W3
    )
    return bass_utils.run_bass_kernel_spmd(
        nc, in_maps, core_ids=list(range(N_CORES)), trace=True,
        trace_cores=trace_cores, tmpdir=tmpdir,
    )



# revision 7
# speedup vs baseline: 2.7135x; 2.7135x over previous
"""Trainium2 Bass kernel for nn_MinimalNetwork (equivariant GNN message passing).

v2 design, sharded over 8 NeuronCores by contiguous edge ranges:
  host: gathers F = features[src] (col-permuted, fp16), transposes rsh (fp16),
        permutes W3 / CC2 columns into kernel-friendly layouts.
  device, per 512-edge supertile (4 chunks x 128 edges on partitions):
    radial basis (ScalarE) -> 3-layer fp16 MLP (TensorE) -> R = h @ W3p (fp16)
    CY = rshT^T @ CC2 (TensorE, fp16)
    G = F (x) CY outer products (VectorE, fp16)
    D = sum_ii G      -- identity-stationary matmuls accumulating in PSUM
    Q = R * D products (VectorE fp16 2x mode, 6-dim APs)
    msg = sum_{j,v,k} Q -- two identity-matmul stages (sum k+j, then sum v)
    per-edge messages DMA'd out; NO gather/scatter on device.
  host: segment-sum per-edge messages by dst (scipy.sparse / np.add.at).

Self-contained: shapes hardcoded for the 200000-edge / 12500-node instance.
"""

import math
from contextlib import ExitStack
from itertools import accumulate

import numpy as np

# ----------------- problem constants (hardcoded) -----------------
N_NODES = 12500
N_EDGES = 200000
N_CORES = 8
SH_DIM = 25
N_BASIS, H = 10, 100
MIN_R, MAX_R = 0.7, 3.2
SWISH_SCALE = 1.679177
SUB = 128
SUPER = 512
N_SUB = SUPER // SUB

NO = [1, 3, 5]                      # 2*lo+1
NJ = [1, 3, 5]                      # 2*lj+1


def _nl(i, j):
    return 2 * min(i, j) + 1


W_J = [sum(NO[i] * _nl(i, j) for i in range(3)) for j in range(3)]  # [9,25,35]


def _wsect(i, j):
    return sum(NO[i2] * _nl(i2, j) for i2 in range(i))


FEAT_OFF = [0, 8, 32, 72]           # reference feature layout (j, v, ii)
FOFF = [0, 8, 32, 72]               # kernel F layout (j, ii, v)
CYOFF = [0] + list(accumulate(NJ[j] * W_J[j] for j in range(3)))  # [0,9,84,259]
CY_DIM = CYOFF[-1]                  # 259
R_OFF = [0] + list(
    accumulate(64 * _nl(i, j) for i in range(3) for j in range(3))
)
R_DIM = R_OFF[-1]                   # 1216
DOFF = [0] + list(accumulate(8 * W_J[j] for j in range(3)))  # [0,72,272,552]
D_DIM = DOFF[-1]                    # 552
G_JOFF = [0, NJ[1] * W_J[1] * 8]    # within g_t chunk: j1 at 0 (600), j2 at 600
G_DIM = G_JOFF[1] + NJ[2] * W_J[2] * 8   # 2000
I12 = [(1, 0), (1, 1), (1, 2), (2, 0), (2, 1), (2, 2)]
QOFF = {}
_q = 0
for (i, j) in I12:
    QOFF[(i, j)] = _q
    _q += 64 * _nl(i, j) * NO[i]
Q_DIM = _q                          # 4224
Q0_DIM = 192
MOFF = [0, 8, 32]                   # msg psum col offset per i (u*no+o inside)
MS_OFF = [0, 192]                   # m_sb sections: i1 [0:192], i2 [192:512]
MS_DIM = 512


def _cc_layout():
    layout, off = {}, 0
    for lo in range(3):
        for li in range(3):
            for lf in range(abs(lo - li), lo + li + 1):
                if (lo, li, lf) not in layout:
                    shp = (2 * lo + 1, 2 * li + 1, 2 * lf + 1)
                    layout[(lo, li, lf)] = (off, shp)
                    off += shp[0] * shp[1] * shp[2]
    return layout, off


CC_LAYOUT, CC_TOTAL = _cc_layout()  # 1225


def _norm_coef():
    nc = np.zeros((3, 3), dtype=np.float64)
    for i in range(3):
        ns = sum(8 * _nl(i, j) for j in range(3))
        nc[i, :] = math.sqrt(4 * math.pi) * math.sqrt(2 * i + 1) / math.sqrt(ns)
    return nc


NORM = _norm_coef()


# ----------------- host-side constant builders -----------------

def build_cc2(cc):
    """CC2 [25, 259]; CY[e, CYOFF[j]+ii*W_J[j]+wsect(i,j)+o*nl+k] =
    sum_f rsh[e, lf^2+f] * NORM[i,j] * C[o, ii, f],  lf = |i-j|+k."""
    cc2 = np.zeros((SH_DIM, CY_DIM), dtype=np.float32)
    for j in range(3):
        for ii in range(NJ[j]):
            for i in range(3):
                nl = _nl(i, j)
                base = CYOFF[j] + ii * W_J[j] + _wsect(i, j)
                for k, lf in enumerate(range(abs(i - j), i + j + 1)):
                    off, shp = CC_LAYOUT[(i, j, lf)]
                    C = cc[off: off + shp[0] * shp[1] * shp[2]].reshape(shp)
                    for o in range(NO[i]):
                        col = base + k * NO[i] + o
                        cc2[lf * lf: lf * lf + 2 * lf + 1, col] = (
                            np.float32(NORM[i, j]) * C[o, ii, :]
                        )
    return cc2


def permute_w3(W3f):
    """W3f [100, 1216] (scales folded) -> kernel column order.
    orig col (i,j)-block: R_OFF[p] + u*(8*nl) + v*nl + k
    new  col: i=0: R_OFF[p] + v*8 + u ; i>=1: R_OFF[p] + k*64 + u*8 + v."""
    perm = np.zeros(R_DIM, dtype=np.int64)
    for i in range(3):
        for j in range(3):
            p = i * 3 + j
            nl = _nl(i, j)
            for u in range(8):
                for v in range(8):
                    for k in range(nl):
                        orig = R_OFF[p] + u * (8 * nl) + v * nl + k
                        if i == 0:
                            new = R_OFF[p] + v * 8 + u
                        else:
                            new = R_OFF[p] + k * 64 + u * 8 + v
                        perm[new] = orig
    return np.ascontiguousarray(W3f[:, perm])


def feat_perm():
    """col perm: orig (j, v, ii) -> new (j, ii, v)."""
    perm = np.zeros(72, dtype=np.int64)
    for j in range(3):
        for v in range(8):
            for ii in range(NJ[j]):
                orig = FEAT_OFF[j] + v * NJ[j] + ii
                new = FOFF[j] + ii * 8 + v
                perm[new] = orig
    return perm


def fold_weights(W0, W1, W2, W3):
    s = SWISH_SCALE
    return (
        (W0 / math.sqrt(N_BASIS)).astype(np.float32),
        (s * W1 / math.sqrt(H)).astype(np.float32),
        (s * W2 / math.sqrt(H)).astype(np.float32),
        (s * W3 / math.sqrt(H)).astype(np.float32),
    )


# ----------------- numpy emulation (layout validation) -----------------

def emulate_core(Fp, rsh, radii, cc2, W0p, W1p, W2p, W3p):
    """Emulate the device pipeline in fp32 for E edges.
    Fp: [E, 72] permuted features; returns msg [E, 72] in reference layout."""
    E = Fp.shape[0]
    centers = np.linspace(MIN_R, MAX_R, N_BASIS).astype(np.float32)
    spacing = (MAX_R - MIN_R) / (N_BASIS - 1)
    z = (radii[:, None] - centers) / spacing
    bas = np.exp(-(z ** 2))
    silu = lambda x: x / (1.0 + np.exp(-x))
    h = silu(bas @ W0p)
    h = silu(h @ W1p)
    h = silu(h @ W2p)
    R = h @ W3p                                     # [E, 1216] kernel layout
    CY = rsh @ cc2                                  # [E, 259]
    # G / D
    D = np.zeros((E, D_DIM), dtype=np.float32)
    for j in range(3):
        Fj = Fp[:, FOFF[j]:FOFF[j + 1]].reshape(E, NJ[j], 8)
        CYj = CY[:, CYOFF[j]:CYOFF[j + 1]].reshape(E, NJ[j], W_J[j])
        Dj = np.einsum("eiv,eiw->ewv", Fj, CYj)     # [E, W_j, 8] w-major
        D[:, DOFF[j]:DOFF[j + 1]] = Dj.reshape(E, -1)
    # Q + sums
    msg = np.zeros((E, 72), dtype=np.float32)
    for i in range(3):
        no = NO[i]
        acc = np.zeros((E, 8, no), dtype=np.float32)
        for j in range(3):
            p = i * 3 + j
            nl = _nl(i, j)
            Rb = R[:, R_OFF[p]:R_OFF[p + 1]]
            Dj = D[:, DOFF[j]:DOFF[j + 1]].reshape(E, W_J[j], 8)
            Dsect = Dj[:, _wsect(i, j):_wsect(i, j) + no * nl, :].reshape(
                E, nl, no, 8)
            if i == 0:
                Rb = Rb.reshape(E, 8, 8)            # [v, u]
                acc[:, :, 0] += np.einsum("evu,ev->eu", Rb, Dsect[:, 0, 0, :])
            else:
                Rb = Rb.reshape(E, nl, 8, 8)        # [k, u, v]
                acc += np.einsum("ekuv,ekov->euo", Rb, Dsect)
        msg[:, MOFF[i]:MOFF[i] + 8 * no] = acc.reshape(E, 8 * no)
    return msg


# ----------------- bass program -----------------

def build_program(e_pad: int):
    import concourse.tile as tile
    from concourse import bacc, mybir
    from concourse.masks import make_identity

    f32 = mybir.dt.float32
    f16 = mybir.dt.float16
    AF = mybir.ActivationFunctionType
    OP = mybir.AluOpType

    n_super = e_pad // SUPER
    assert e_pad % SUPER == 0

    nc = bacc.Bacc()

    rshT_d = nc.dram_tensor("rshT", [SH_DIM, e_pad], f16, kind="ExternalInput")
    radii_d = nc.dram_tensor("radii", [1, e_pad], f32, kind="ExternalInput")
    fg_d = nc.dram_tensor("fg", [n_super * SUB, N_SUB * 72], f16,
                          kind="ExternalInput")
    w0_d = nc.dram_tensor("W0p", [N_BASIS, H], f16, kind="ExternalInput")
    w1_d = nc.dram_tensor("W1p", [H, H], f16, kind="ExternalInput")
    w2_d = nc.dram_tensor("W2p", [H, H], f16, kind="ExternalInput")
    w3_d = nc.dram_tensor("W3p", [H, R_DIM], f16, kind="ExternalInput")
    cc2_d = nc.dram_tensor("CC2", [SH_DIM, CY_DIM], f16, kind="ExternalInput")
    csc_d = nc.dram_tensor("cscale", [N_BASIS, 1], f32, kind="ExternalInput")
    cbi_d = nc.dram_tensor("cbias", [N_BASIS, 1], f32, kind="ExternalInput")
    out_d = nc.dram_tensor("msg", [e_pad, 72], f32, kind="ExternalOutput")

    with tile.TileContext(nc) as tc, ExitStack() as ctx:
        cpool = ctx.enter_context(tc.tile_pool(name="consts", bufs=1))
        inpool = ctx.enter_context(tc.tile_pool(name="in", bufs=3))
        hpool = ctx.enter_context(tc.tile_pool(name="h", bufs=2))
        spool = ctx.enter_context(tc.tile_pool(name="sup", bufs=2))
        mpool = ctx.enter_context(tc.tile_pool(name="m", bufs=3))
        ps_h = ctx.enter_context(tc.tile_pool(name="psh", bufs=1, space="PSUM"))
        ps_acc = ctx.enter_context(tc.tile_pool(name="psacc", bufs=2, space="PSUM"))
        ps_d = ctx.enter_context(tc.tile_pool(name="psd", bufs=2, space="PSUM"))
        ps_m = ctx.enter_context(tc.tile_pool(name="psm", bufs=2, space="PSUM"))
        ps_msg = ctx.enter_context(tc.tile_pool(name="psmsg", bufs=1, space="PSUM"))

        w0_s = cpool.tile([N_BASIS, H], f16)
        w1_s = cpool.tile([H, H], f16)
        w2_s = cpool.tile([H, H], f16)
        w3_s = cpool.tile([H, R_DIM], f16)
        cc2_s = cpool.tile([SH_DIM, CY_DIM], f16)
        csc_s = cpool.tile([N_BASIS, 1], f32)
        cbi_s = cpool.tile([N_BASIS, 1], f32)
        ident = cpool.tile([SUB, SUB], f16)
        for t, d in (
            (w0_s, w0_d), (w1_s, w1_d), (w2_s, w2_d), (w3_s, w3_d),
            (cc2_s, cc2_d), (csc_s, csc_d), (cbi_s, cbi_d),
        ):
            nc.sync.dma_start(t[:], d[:])
        make_identity(nc, ident[:])

        for s in range(n_super):
            e0 = s * SUPER
            # ---- input loads ----
            rshT_t = inpool.tile([SH_DIM, SUPER], f16, tag="rsh")
            nc.sync.dma_start(rshT_t[:], rshT_d[:, e0:e0 + SUPER])
            radb = inpool.tile([N_BASIS, SUPER], f32, tag="radb")
            nc.sync.dma_start(
                radb[:], radii_d[:, e0:e0 + SUPER].broadcast_to(
                    (N_BASIS, SUPER)),
            )
            fg_t = inpool.tile([SUB, N_SUB * 72], f16, tag="fg")
            nc.sync.dma_start(fg_t[:], fg_d[s * SUB:(s + 1) * SUB, :])

            # ---- radial basis (ScalarE) ----
            z2_t = hpool.tile([N_BASIS, SUPER], f32, tag="z2")
            nc.scalar.activation(z2_t[:], radb[:], AF.Square,
                                 bias=cbi_s[:], scale=csc_s[:])
            bas_t = hpool.tile([N_BASIS, SUPER], f16, tag="bas")
            nc.scalar.activation(bas_t[:], z2_t[:], AF.Exp, scale=-1.0)

            # ---- MLP (fp16) ----
            hcur = bas_t
            for li, w_s in enumerate((w0_s, w1_s, w2_s)):
                hp = ps_h.tile([H, SUPER], f32, tag="hp", space="PSUM")
                nc.tensor.matmul(hp[:], w_s[:], hcur[:], start=True, stop=True)
                hn = hpool.tile([H, SUPER], f16, tag=f"h{li}")
                nc.scalar.activation(hn[:], hp[:], AF.Silu)
                hcur = hn

            # ---- per-supertile work tiles ----
            r_sb = spool.tile([SUB, N_SUB * R_DIM], f16, tag="rsb")
            cy_sb = spool.tile([SUB, N_SUB * CY_DIM], f16, tag="cysb")
            g_t = spool.tile([SUB, N_SUB * G_DIM], f16, tag="g")
            d_sb = spool.tile([SUB, N_SUB * D_DIM], f16, tag="d")
            q_t = spool.tile([SUB, N_SUB * Q_DIM], f16, tag="q")
            q0_t = spool.tile([SUB, N_SUB * Q0_DIM], f16, tag="q0")

            for c in range(N_SUB):
                csl = slice(c * SUB, (c + 1) * SUB)
                # ---- R = h3_c^T @ W3p ----
                for n0 in range(0, R_DIM, 512):
                    n1 = min(n0 + 512, R_DIM)
                    r_ps = ps_acc.tile([SUB, 512], f32, tag="acc", space="PSUM")
                    nc.tensor.matmul(r_ps[:, : n1 - n0], hcur[:, csl],
                                     w3_s[:, n0:n1], start=True, stop=True)
                    nc.scalar.activation(
                        r_sb[:, c * R_DIM + n0: c * R_DIM + n1],
                        r_ps[:, : n1 - n0], AF.Copy)
                # ---- CY ----
                cy_ps = ps_acc.tile([SUB, 512], f32, tag="acc", space="PSUM")
                nc.tensor.matmul(cy_ps[:, :CY_DIM], rshT_t[:, csl], cc2_s[:],
                                 start=True, stop=True)
                nc.scalar.activation(
                    cy_sb[:, c * CY_DIM:(c + 1) * CY_DIM], cy_ps[:, :CY_DIM],
                    AF.Copy)

            fg3 = fg_t[:].rearrange("p (c f) -> p c f", c=N_SUB)
            cy3 = cy_sb[:].rearrange("p (c f) -> p c f", c=N_SUB)
            g3 = g_t[:].rearrange("p (c f) -> p c f", c=N_SUB)
            d3 = d_sb[:].rearrange("p (c f) -> p c f", c=N_SUB)
            r3 = r_sb[:].rearrange("p (c f) -> p c f", c=N_SUB)
            q3 = q_t[:].rearrange("p (c f) -> p c f", c=N_SUB)
            q03 = q0_t[:].rearrange("p (c f) -> p c f", c=N_SUB)

            # ---- G products (DVE, c-fused) ----
            # j = 0: D_j0 directly: out [c, w(9), v(8)]
            nc.vector.tensor_tensor(
                d3[:, :, 0:72].rearrange("p c (w v) -> p c w v", v=8),
                fg3[:, :, FOFF[0]:FOFF[0] + 8].unsqueeze(2)
                .broadcast_to((SUB, N_SUB, 9, 8)),
                cy3[:, :, CYOFF[0]:CYOFF[0] + 9].unsqueeze(3)
                .broadcast_to((SUB, N_SUB, 9, 8)),
                OP.mult,
            )
            for j in (1, 2):
                nj, wj = NJ[j], W_J[j]
                for ii in range(nj):
                    go = G_JOFF[j - 1] + ii * wj * 8
                    nc.vector.tensor_tensor(
                        g3[:, :, go:go + wj * 8]
                        .rearrange("p c (w v) -> p c w v", v=8),
                        fg3[:, :, FOFF[j] + ii * 8:FOFF[j] + (ii + 1) * 8]
                        .unsqueeze(2).broadcast_to((SUB, N_SUB, wj, 8)),
                        cy3[:, :, CYOFF[j] + ii * wj:CYOFF[j] + (ii + 1) * wj]
                        .unsqueeze(3).broadcast_to((SUB, N_SUB, wj, 8)),
                        OP.mult,
                    )

            # ---- D-sum (TensorE identity matmuls, per chunk) ----
            for c in range(N_SUB):
                for j in (1, 2):
                    nj, wj = NJ[j], W_J[j]
                    m = wj * 8
                    gc = g3[:, c, G_JOFF[j - 1]:G_JOFF[j - 1] + nj * m]
                    dp = ps_d.tile([SUB, 280], f32, tag="dp", space="PSUM")
                    d_ps = dp[:, :m]
                    nc.tensor.matmul(d_ps, ident[:], gc[0:SUB, 0:m],
                                     start=True, stop=(nj == 1))
                    if j == 1:
                        nc.tensor.matmul(
                            d_ps.unsqueeze(1).broadcast_to((SUB, nj - 1, m)),
                            ident[:],
                            gc[:, m:].rearrange("p (i m) -> p i m", i=nj - 1),
                            start=False, stop=True,
                        )
                    else:
                        for ii in range(1, nj):
                            nc.tensor.matmul(
                                d_ps, ident[:], gc[:, ii * m:(ii + 1) * m],
                                start=False, stop=(ii == nj - 1),
                            )
                    nc.scalar.activation(
                        d3[:, c, DOFF[j]:DOFF[j + 1]], d_ps, AF.Copy)

            # ---- Q products (DVE, c-fused) ----
            # i = 0 (per j, 1x): out [c, v, u]
            for j in range(3):
                nc.vector.tensor_tensor(
                    q03[:, :, j * 64:(j + 1) * 64]
                    .rearrange("p c (v u) -> p c v u", v=8),
                    r3[:, :, R_OFF[j]:R_OFF[j] + 64]
                    .rearrange("p c (v u) -> p c v u", v=8),
                    d3[:, :, DOFF[j]:DOFF[j] + 8].unsqueeze(3)
                    .broadcast_to((SUB, N_SUB, 8, 8)),
                    OP.mult,
                )
            # i = 1, 2 (2x mode): out [k, u, o, v]; per chunk (5-D AP limit)
            for c in range(N_SUB):
                for (i, j) in I12:
                    p = i * 3 + j
                    nl, no = _nl(i, j), NO[i]
                    ws = _wsect(i, j)
                    nc.vector.tensor_tensor(
                        q3[:, c, QOFF[(i, j)]:QOFF[(i, j)] + 64 * nl * no]
                        .rearrange("p (k u o v) -> p k u o v", k=nl, u=8, o=no),
                        r3[:, c, R_OFF[p]:R_OFF[p + 1]]
                        .rearrange("p (k u v) -> p k u v", k=nl, u=8)
                        .unsqueeze(3).broadcast_to((SUB, nl, 8, no, 8)),
                        d3[:, c, DOFF[j] + ws * 8: DOFF[j] + (ws + no * nl) * 8]
                        .rearrange("p (k o v) -> p k o v", k=nl, o=no)
                        .unsqueeze(2).broadcast_to((SUB, nl, 8, no, 8)),
                        OP.mult,
                    )

            # ---- stage1 + stage2 sums (TensorE), msg out ----
            for c in range(N_SUB):
                # stage1: per i in (1,2), sum over (j, k) into M psum
                m_sb = mpool.tile([SUB, MS_DIM], f16, tag="msb")
                for ei, i in enumerate((1, 2)):
                    no = NO[i]
                    cols = 64 * no
                    mp = ps_m.tile([SUB, 320], f32, tag="mp", space="PSUM")
                    m_ps = mp[:, :cols]
                    first = True
                    for j in range(3):
                        nl = _nl(i, j)
                        qb = QOFF[(i, j)]
                        for k in range(nl):
                            nc.tensor.matmul(
                                m_ps, ident[:],
                                q3[:, c, qb + k * cols: qb + (k + 1) * cols],
                                start=first,
                                stop=(j == 2 and k == nl - 1),
                            )
                            first = False
                    # evac [u,o,v] -> m_sb [v, u, o]
                    nc.scalar.activation(
                        m_sb[:, MS_OFF[ei]:MS_OFF[ei] + cols]
                        .rearrange("p (v m) -> p v m", v=8).transpose((0, 2, 1)),
                        m_ps.rearrange("p (m v) -> p m v", v=8),
                        AF.Copy,
                    )
                # msg accumulation: i0 then stage2 for i1, i2
                msg_ps = ps_msg.tile([SUB, 72], f32, tag="msg", space="PSUM")
                q0c = q03[:, c, :]
                nc.tensor.matmul(msg_ps[:, 0:8], ident[:], q0c[:, 0:8],
                                 start=True, stop=False)
                nc.tensor.matmul(
                    msg_ps[:, 0:8].unsqueeze(1).broadcast_to((SUB, 23, 8)),
                    ident[:],
                    q0c[:, 8:].rearrange("p (b u) -> p b u", u=8),
                    start=False, stop=True,
                )
                for ei, i in enumerate((1, 2)):
                    no = NO[i]
                    cols = 64 * no
                    m3v = (m_sb[:, MS_OFF[ei]:MS_OFF[ei] + cols]
                           .rearrange("p (v m) -> p v m", v=8))
                    nc.tensor.matmul(
                        msg_ps[:, MOFF[i]:MOFF[i] + 8 * no], ident[:],
                        m3v[:, 0, :], start=True, stop=False,
                    )
                    nc.tensor.matmul(
                        msg_ps[:, MOFF[i]:MOFF[i] + 8 * no]
                        .unsqueeze(1).broadcast_to((SUB, 7, 8 * no)),
                        ident[:], m3v[:, 1:, :], start=False, stop=True,
                    )
                msg_sb = mpool.tile([SUB, 72], f32, tag="msgsb")
                nc.scalar.activation(msg_sb[:], msg_ps[:], AF.Copy)
                nc.sync.dma_start(out_d[e0 + c * SUB: e0 + (c + 1) * SUB, :],
                                  msg_sb[:])

    nc.finalize()
    return nc


# ----------------- host side -----------------

def _prep_consts(cc, W0, W1, W2, W3):
    W0p, W1p, W2p, W3f = fold_weights(
        np.asarray(W0, np.float32), np.asarray(W1, np.float32),
        np.asarray(W2, np.float32), np.asarray(W3, np.float32))
    W3p = permute_w3(W3f)
    cc2 = build_cc2(np.asarray(cc, dtype=np.float32))
    centers = np.linspace(MIN_R, MAX_R, N_BASIS).astype(np.float32)
    spacing = (MAX_R - MIN_R) / (N_BASIS - 1)
    cscale = np.full((N_BASIS, 1), 1.0 / spacing, dtype=np.float32)
    cbias = (-centers / spacing).astype(np.float32).reshape(N_BASIS, 1)
    return W0p, W1p, W2p, W3p, cc2, cscale, cbias


def _build_and_maps(edge_index, features, radii, rsh, cc, W0, W1, W2, W3):
    edge_index = np.asarray(edge_index)
    features = np.asarray(features, dtype=np.float32)
    radii = np.asarray(radii, dtype=np.float32)
    rsh = np.asarray(rsh, dtype=np.float32)
    E = radii.shape[0]
    per_core = E // N_CORES
    assert per_core * N_CORES == E
    n_super = -(-per_core // SUPER)
    e_pad = n_super * SUPER

    W0p, W1p, W2p, W3p, cc2, cscale, cbias = _prep_consts(cc, W0, W1, W2, W3)
    fperm = feat_perm()
    feat_p = np.ascontiguousarray(features[:, fperm]).astype(np.float16)
    src = edge_index[0].astype(np.int64)
    F_all = feat_p[src]                                # [E, 72] fp16

    consts = dict(
        W0p=W0p.astype(np.float16), W1p=W1p.astype(np.float16),
        W2p=W2p.astype(np.float16), W3p=W3p.astype(np.float16),
        CC2=cc2.astype(np.float16), cscale=cscale, cbias=cbias,
    )

    nc = build_program(e_pad)
    in_maps = []
    for kcore in range(N_CORES):
        sl = slice(kcore * per_core, (kcore + 1) * per_core)
        rshT = np.zeros((SH_DIM, e_pad), dtype=np.float16)
        rshT[:, :per_core] = rsh[sl].T.astype(np.float16)
        rad = np.ones((1, e_pad), dtype=np.float32)
        rad[0, :per_core] = radii[sl]
        Fc = np.zeros((e_pad, 72), dtype=np.float16)
        Fc[:per_core] = F_all[sl]
        # [e] -> [s, c, p] -> fg rows [s, p, c*72]
        fg = np.ascontiguousarray(
            Fc.reshape(n_super, N_SUB, SUB, 72).transpose(0, 2, 1, 3)
            .reshape(n_super * SUB, N_SUB * 72))
        in_maps.append(dict(rshT=rshT, radii=rad, fg=fg, **consts))
    return nc, in_maps, per_core, e_pad


def _combine(msgs, dst, n_nodes):
    out = np.zeros((n_nodes, 72), dtype=np.float32)
    try:
        from scipy.sparse import csr_matrix
        E = dst.shape[0]
        S = csr_matrix(
            (np.ones(E, np.float32), (dst, np.arange(E))), shape=(n_nodes, E))
        out += S @ msgs
    except ImportError:
        np.add.at(out, dst, msgs)
    return out


def kernel(edge_index, features, radii, rsh, cc, W0, W1, W2, W3):
    from concourse import bass_utils

    nc, in_maps, per_core, e_pad = _build_and_maps(
        edge_index, features, radii, rsh, cc, W0, W1, W2, W3)
    res = bass_utils.run_bass_kernel_spmd(
        nc, in_maps, core_ids=list(range(N_CORES)))
    msgs = np.concatenate(
        [r["msg"][:per_core] for r in res.results], axis=0)
    dst = np.asarray(edge_index)[1].astype(np.int64)
    return _combine(msgs.astype(np.float32), dst, N_NODES)


def _install_ntff_shim():
    """Provide antenv.axon_hooks + the ctypes NTFF hook if absent."""
    import contextlib
    import ctypes
    import sys
    import types

    try:
        from antenv.axon_hooks import get_axon_ntff_profile_hook  # noqa: F401
        return
    except ImportError:
        pass

    holder = {}
    mod = types.ModuleType("antenv.axon_hooks")
    mod.set_axon_ntff_profile_hook = lambda h: holder.__setitem__("h", h)
    mod.get_axon_ntff_profile_hook = lambda: holder.get("h")
    import antenv

    sys.modules["antenv.axon_hooks"] = mod
    antenv.axon_hooks = mod

    so_path = "/opt/axon/libaxon_pjrt.so"
    try:
        lib = ctypes.CDLL(so_path)
    except OSError:
        return
    if not hasattr(lib, "axon_start_nrt_profile"):
        return
    lib.axon_start_nrt_profile.argtypes = [
        ctypes.POINTER(ctypes.c_int64),
        ctypes.c_size_t,
    ]
    lib.axon_start_nrt_profile.restype = ctypes.c_int64
    lib.axon_stop_nrt_profile.argtypes = [ctypes.c_char_p]
    lib.axon_stop_nrt_profile.restype = ctypes.c_int64

    @contextlib.contextmanager
    def _hook(output_dir, device_ids):
        import jax

        jax.devices()
        if device_ids:
            ids = (ctypes.c_int64 * len(device_ids))(*device_ids)
            rc = lib.axon_start_nrt_profile(ids, len(device_ids))
        else:
            rc = lib.axon_start_nrt_profile(None, 0)
        if rc != 0:
            raise RuntimeError(f"axon_start_nrt_profile rc={rc}")
        try:
            yield
        finally:
            n = lib.axon_stop_nrt_profile(str(output_dir).encode())
            print(f"ntff profile: {n} file(s) written to {output_dir}")

    mod.set_axon_ntff_profile_hook(_hook)


def kernel_traced(edge_index, features, radii, rsh, cc, W0, W1, W2, W3,
                  trace_cores=None, tmpdir=None):
    """Run with NTFF tracing; returns BassKernelResults."""
    _install_ntff_shim()
    from concourse import bass_utils

    bass_utils.upload_artifacts = lambda d: f"local:{d}"

    nc, in_maps, per_core, e_pad = _build_and_maps(
        edge_index, features, radii, rsh, cc, W0, W1, W2, W3)
    return bass_utils.run_bass_kernel_spmd(
        nc, in_maps, core_ids=list(range(N_CORES)), trace=True,
        trace_cores=trace_cores, tmpdir=tmpdir,
    )
